# revision 37
# baseline (speedup 1.0000x reference)
"""Trainium2 Bass kernel for LIFNet (leaky-integrator net, no spiking).

Math: the module is linear, and the leaky integration L (a causal LTI filter
along T) commutes with the per-timestep linear layers:

    V2 = L(L(batch @ W1^T) @ W2^T) = (L^2)(batch @ (W2 @ W1)^T)

with Wc = W2 @ W1 of shape [10, 784].  L^2 has impulse response
h[m] = beta^2 (m-1) alpha^(m-2) (m >= 2), which decays below f32 noise by
lag ~128, so the filter is applied as a banded blocked matmul with two
constant 128x128 blocks (intra-block R0, previous-block R1).

The kernel is HBM-bandwidth-bound (the batch read dominates), so:
  - x is pre-cast to fp16 on the host (quantization adds ~4e-4 rel err
    against a 2e-2 gate), halving DMA bytes vs f32.
  - The DRAM layout is per-partition contiguous per (b, t-half), giving
    14 KB descriptors; measured fastest issue pattern under all-8-core
    load is 26 half-b DMAs on the sync HWDGE queue (~250 GB/s/core; the
    f32 4 KB-descriptor version ran at ~207 GB/s).

Device work per core (13 b's, data-parallel over batch; groups of 4 b's
packed 32-partitions apart so downstream stages run 4 b's per instruction):
  - z^T = Wc @ x^T via PE matmuls (fp16, Wc chunks [112, 32] zero-padded,
    tile_position=(0, 32i) places b_i's output rows at psum partition 32i).
  - zp [128, 500] f32 -> zts [128, 2048] fp16 per-(b, tg) cast copies
    (scalar engine) issued right behind each b's matmul burst, so the
    next group's matmuls are not gated on a group-end copy barrier.
  - PE transpose per 128-t-block: [128, 128] -> tpsum fp16; DVE compacts
    the 4x10 used columns into zb slabs [128, 40].
  - V2^T[4 b's] per t'-block via two K=128 fp16 matmuls (R1 prev / R0 cur).
  - v2 [40, 2000] f32 DMA'd out per group on the scalar HWDGE queue.

PE-clock (HAM) management: the tensor engine re-throttles to 1.2 GHz
after any ~3.4us idle window, and transpose-mode matmuls do not count
as activity.  A few dummy matmuls pad each half-b burst and the
transpose bursts so the clock stays at 2.4 GHz.

Tail: the 1-b group is loaded and processed FIRST (its whole pipeline
hides under the other groups' loads); each group's t'-blocks 0-6 are
transposed + filtered as soon as its last b's first half lands; the
last group uses per-b transposes so after the final DMA byte only the
final b's own transposes + the filter remain serial.  Outputs are fp16
(host upcasts) and split per group into [0:896) / [896:2000) slices so
most of the write overlaps the filter.
"""

import sys

import numpy as np

for _p in ("/opt/trn_rl_repo",):
    if _p not in sys.path:
        sys.path.append(_p)

B, T, DIN, H1, H2 = 100, 2000, 784, 100, 10
ALPHA, BETA = 0.7, 0.3

NCORES = 8
BPAD = 104           # batch padded to 8 * 13
BP = BPAD // NCORES  # 13 b's per core
DC = 112             # d-chunk width (784 = 7 * 112), partition dim of x tiles
NDC = DIN // DC      # 7
MP = 32              # padded output rows per b (10 real + 22 zero)
TH = T // 2          # t-half
TG = 500             # t-columns per z-matmul group (one psum bank)
NTG = T // TG        # 4
TB = 128             # t'-block for the filter stage
NTB = (T + TB - 1) // TB   # 16
TPADF = NTB * TB     # 2048 free-dim padding for the z^T staging buffer
# Singleton first: its data arrives in the first two DMA slots and its
# whole pipeline overlaps the other groups' loads, so the kernel tail is
# only the last 4-group's second filter phase.
GROUPS = [(12, 1), (0, 4), (4, 4), (8, 4)]  # (first b, group size)
NWARM = 4            # dummy N=500 matmuls appended per half-b burst

_CACHE: dict = {}


def _filter_blocks() -> np.ndarray:
    """R = [R1 | R0] as [128, 256] fp16: rhs blocks for the filter matmuls.

    out[o, t'] += sum_tl z_block[tl, o] * R[tl, t'] with R[tl, t'] =
    h[lag], lag = (t' - tl) + 128 for R1 (z from previous t-block) and
    (t' - tl) for R0 (intra-block, strictly causal).
    """
    m = np.arange(512, dtype=np.float64)
    h = np.zeros(512)
    h[2:] = BETA * BETA * (m[2:] - 1.0) * ALPHA ** (m[2:] - 2.0)
    tl = np.arange(TB)[:, None]
    tp = np.arange(TB)[None, :]
    r1 = h[tp - tl + TB]
    lag0 = tp - tl
    r0 = np.where(lag0 >= 2, h[np.clip(lag0, 0, None)], 0.0)
    return np.concatenate([r1, r0], axis=1).astype(np.float16)


def _build(reps: int = 1):
    """Build + compile the per-core Bass kernel (shared by all 8 cores)."""
    from contextlib import ExitStack

    import concourse.tile as tile
    from concourse import bacc, mybir

    f16 = mybir.dt.float16
    f32 = mybir.dt.float32
    nc = bacc.Bacc(
        "TRN2", target_bir_lowering=False, debug=False, num_devices=NCORES
    )

    # per-partition layout per b: [half h][chunk c][t' in half] (7000 each)
    xT = nc.dram_tensor("xT", [BP, DC, NDC * T], f16, kind="ExternalInput")
    wct = nc.dram_tensor("wct", [DC, NDC * MP], f16, kind="ExternalInput")
    rh = nc.dram_tensor("rh", [TB, 2 * TB], f16, kind="ExternalInput")
    eye = nc.dram_tensor("eye", [TB, TB], f16, kind="ExternalInput")
    vout = nc.dram_tensor("vout", [BP * H2, T], f16, kind="ExternalOutput")

    with tile.TileContext(nc) as tc, ExitStack() as ctx:
        const = ctx.enter_context(tc.tile_pool(name="const", bufs=1))
        xpool = ctx.enter_context(tc.tile_pool(name="xp", bufs=10))
        ring = ctx.enter_context(tc.tile_pool(name="ring", bufs=1))
        zbp = ctx.enter_context(tc.tile_pool(name="zbp", bufs=2))
        vsb = ctx.enter_context(tc.tile_pool(name="vsb", bufs=2))
        zpsum = ctx.enter_context(tc.tile_pool(name="zps", bufs=1, space="PSUM"))
        tpsum = ctx.enter_context(tc.tile_pool(name="tps", bufs=2, space="PSUM"))
        vpsum = ctx.enter_context(tc.tile_pool(name="vps", bufs=1, space="PSUM"))
        dpsum = ctx.enter_context(tc.tile_pool(name="dps", bufs=1, space="PSUM"))

        # consts on the scalar HWDGE queue so they don't delay the first
        # x load on the sync queue
        wct_sb = const.tile([DC, NDC * MP], f16, tag="wct")
        nc.scalar.dma_start(wct_sb[:], wct.ap())
        rh_sb = const.tile([TB, 2 * TB], f16, tag="rh")
        nc.scalar.dma_start(rh_sb[:], rh.ap())
        eye_sb = const.tile([TB, TB], f16, tag="eye")
        nc.scalar.dma_start(eye_sb[:], eye.ap())

        # Two-deep manual ring: the t-pad cols (>=2000) of the z^T staging
        # tile must stay zero across groups, so memset only once.
        zts_ring = []
        for i in range(2):
            zt = ring.tile([TB, TPADF], f16, tag=f"zts{i}", name=f"zts{i}")
            nc.vector.memset(zt[:], 0.0)
            zts_ring.append(zt)

        def warm(xv, n=NWARM):
            """Dummy matmuls: count as PE activity for the HAM clock gate."""
            dmy = dpsum.tile([1, TG], f32, tag="dmy", name="dmy")
            for _ in range(n):
                nc.tensor.matmul(
                    dmy[:], wct_sb[:, 0:1], xv[:, 0, 0:TG],
                    start=True, stop=True,
                )

        def z_half(zp_tiles, xv, i, h, copy_rows, zts, nwarm=NWARM):
            """One half-b of stage-1 matmuls + its two zts cast copies."""
            for tg in (0, 1):
                zp = zp_tiles[2 * h + tg]
                for c in range(NDC):
                    nc.tensor.matmul(
                        zp[MP * i : MP * (i + 1), :],
                        wct_sb[:, c * MP : (c + 1) * MP],
                        xv[:, c, tg * TG : (tg + 1) * TG],
                        start=(c == 0),
                        stop=(c == NDC - 1),
                        tile_position=(0, MP * i),
                    )
            warm(xv, nwarm)
            r0, r1 = copy_rows
            for tg in (0, 1):
                gtg = 2 * h + tg
                nc.scalar.copy(
                    zts[r0:r1, gtg * TG : (gtg + 1) * TG],
                    zp_tiles[gtg][r0:r1, :],
                )

        def transposes(zts, zbv, G, xv_warm, j0, j1, per_b=None):
            """z^T -> zb for t'-blocks [j0, j1).

            per_b=i transposes only b_i's 32-partition band (so the last
            group's final b leaves just its own transposes for the tail).
            """
            for j in range(j0, j1):
                tp = tpsum.tile([TB, TB], f16, tag="tp", name="tp")
                if per_b is None:
                    nc.tensor.transpose(
                        tp[:], zts[:, j * TB : (j + 1) * TB], eye_sb[:]
                    )
                    tpv = tp[:].rearrange("p (gg o) -> p gg o", gg=4)
                    nc.vector.tensor_copy(
                        zbv[:, j, 0:G, :], tpv[:, 0:G, 0:H2]
                    )
                else:
                    i = per_b
                    nc.tensor.transpose(
                        tp[:, 0:MP],
                        zts[MP * i : MP * (i + 1), j * TB : (j + 1) * TB],
                        eye_sb[MP * i : MP * (i + 1), MP * i : MP * (i + 1)],
                        tile_position=(MP * i, 0),
                    )
                    nc.vector.tensor_copy(
                        zbv[:, j, i, :], tp[:, 0:H2]
                    )
                if j % 4 == 3 and xv_warm is not None:
                    # transpose-mode matmuls don't register as PE activity
                    # for the clock gate; sprinkle a real one
                    dmy = dpsum.tile([1, TG], f32, tag="dmy", name="dmy")
                    nc.tensor.matmul(
                        dmy[:], wct_sb[:, 0:1], xv_warm[:, 0, 0:TG],
                        start=True, stop=True,
                    )

        def stage23(zts, zb, zbv, v2, G, OG, xv_warm, j0, j1, skip_t=False):
            """Transpose + filter for t'-blocks [j0, j1)."""
            if not skip_t:
                transposes(zts, zbv, G, xv_warm, j0, j1)
            for j in range(j0, j1):
                vp = vpsum.tile([4 * H2, TB], f32, tag="vp", name="vp")
                n_mm = 2 if j > 0 else 1
                mm = 0
                for roff, jj in ((0, j - 1), (TB, j)):
                    if jj < 0:
                        continue
                    nc.tensor.matmul(
                        vp[0:OG, :],
                        zb[:, jj * 4 * H2 : jj * 4 * H2 + OG],
                        rh_sb[:, roff : roff + TB],
                        start=(mm == 0),
                        stop=(mm == n_mm - 1),
                    )
                    mm += 1
                w = min(TB, T - j * TB)
                nc.vector.tensor_copy(
                    v2[0:OG, j * TB : j * TB + w], vp[0:OG, 0:w]
                )

        for rep in range(reps):
          for g, (b0, G) in enumerate(GROUPS):
            zts = zts_ring[g % 2]
            last_grp = g == len(GROUPS) - 1

            zp_tiles = [
                zpsum.tile([TB, TG], f32, tag=f"zp{tg}", name=f"zp{tg}")
                for tg in range(NTG)
            ]
            zb = zbp.tile([TB, NTB * 4 * H2], f16, tag="zb")
            zbv = zb[:].rearrange("p (j gg o) -> p j gg o", j=NTB, gg=4)
            v2 = vsb.tile([4 * H2, T], f16, tag="v2")
            OG = H2 * G

            for i in range(G):
                b = b0 + i
                rows = (MP * i, MP * (i + 1))
                for h in range(2):
                    xt = xpool.tile([DC, NDC * TH], f16, tag="xt")
                    xv = xt[:].rearrange("p (c t) -> p c t", c=NDC)
                    if last_grp and i == G - 1 and h == 1:
                        # the very last load, split by t-group into two
                        # dedicated tiles (tile-granular deps; 1000-B
                        # descriptor runs are slightly slower but tiny):
                        # the serial tail only waits on the final ~0.78 MB,
                        # and blocks 7-10 transpose+filter under it
                        xd = xT.ap()[b].rearrange(
                            "p (hh c t) -> p hh c t", hh=2, c=NDC
                        )
                        for tg in (2, 3):
                            xq = xpool.tile(
                                [DC, NDC * TG], f16, tag=f"xq{tg}",
                                name=f"xq{tg}", bufs=1,
                            )
                            xqv = xq[:].rearrange("p (c t) -> p c t", c=NDC)
                            sl = slice((tg - 2) * TG, (tg - 1) * TG)
                            nc.sync.dma_start(xqv[:, :, :], xd[:, 1, :, sl])
                            zp = zp_tiles[tg]
                            for c in range(NDC):
                                nc.tensor.matmul(
                                    zp[rows[0] : rows[1], :],
                                    wct_sb[:, c * MP : (c + 1) * MP],
                                    xqv[:, c, :],
                                    start=(c == 0),
                                    stop=(c == NDC - 1),
                                    tile_position=(0, MP * i),
                                )
                            nc.scalar.copy(
                                zts[rows[0] : rows[1], tg * TG : (tg + 1) * TG],
                                zp[rows[0] : rows[1], :],
                            )
                            if tg == 2:
                                # blocks 7-10 only need t < 1408
                                transposes(zts, zbv, G, xqv, 7, 11, per_b=i)
                                stage23(zts, zb, zbv, v2, G, OG, xqv, 7, 11,
                                        skip_t=True)
                            else:
                                transposes(zts, zbv, G, xqv, 11, NTB,
                                           per_b=i)
                        continue
                    nc.sync.dma_start(
                        xt[:],
                        xT.ap()[b, :, h * NDC * TH : (h + 1) * NDC * TH],
                    )
                    z_half(zp_tiles, xv, i, h, rows, zts,
                           nwarm=8 if (last_grp and i == G - 1) else NWARM)
                    if i == G - 1 and h == 0:
                        # t'-blocks 0-6 only need t < 896: transpose +
                        # filter them while the last half-b streams in
                        stage23(zts, zb, zbv, v2, G, OG, xv, 0, 7)
                        nc.scalar.dma_start(
                            vout.ap()[H2 * b0 : H2 * b0 + OG, 0 : 7 * TB],
                            v2[0:OG, 0 : 7 * TB],
                        )
                    if last_grp and h == 1:
                        # per-b transposes as each b's second half finishes:
                        # after the final DMA only the last b's own
                        # transposes and filter remain on the critical path
                        transposes(zts, zbv, G, xv, 7, NTB, per_b=i)
            j0_final = 11 if last_grp else 7
            stage23(zts, zb, zbv, v2, G, OG, xv, j0_final, NTB,
                    skip_t=last_grp)
            # final slice on the sync queue: it is idle by now, while the
            # scalar engine is still draining copies ahead of its queue
            outq = nc.sync if g == len(GROUPS) - 1 else nc.scalar
            outq.dma_start(
                vout.ap()[H2 * b0 : H2 * b0 + OG, 7 * TB : T],
                v2[0:OG, 7 * TB : T],
            )

    nc.compile()
    return nc


def _prep_inputs(batch: np.ndarray, W1: np.ndarray, W2: np.ndarray):
    wc = (W2.astype(np.float64) @ W1.astype(np.float64)).astype(np.float32)
    # [112, 7, 32]: wct[p, c, o] = Wc[o, 112c + p] for o < 10, else 0
    wct = np.zeros((DC, NDC, MP), np.float16)
    wct[:, :, :H2] = wc.T.reshape(NDC, DC, H2).transpose(1, 0, 2)
    wct = np.ascontiguousarray(wct.reshape(DC, NDC * MP))
    rh = _filter_blocks()
    eye = np.eye(TB, dtype=np.float16)

    bp = np.zeros((BPAD, T, DIN), np.float16)
    bp[:B] = batch.astype(np.float16)
    # [8, 13, 112, 2, 7, 1000]: per-(b, half) partition runs of 14 KB
    xt = np.ascontiguousarray(
        bp.reshape(NCORES, BP, 2, TH, NDC, DC).transpose(0, 1, 5, 2, 4, 3)
    ).reshape(NCORES, BP, DC, NDC * T)
    return xt, wct, rh, eye


def kernel(batch: np.ndarray, W1: np.ndarray, W2: np.ndarray) -> np.ndarray:
    from concourse import bass_utils

    if "nc" not in _CACHE:
        _CACHE["nc"] = _build()
    nc = _CACHE["nc"]

    xt, wct, rh, eye = _prep_inputs(batch, W1, W2)
    in_maps = [
        {"xT": xt[i], "wct": wct, "rh": rh, "eye": eye} for i in range(NCORES)
    ]
    res = bass_utils.run_bass_kernel_spmd(
        nc, in_maps, core_ids=list(range(NCORES)), **_CACHE.get("run_kwargs", {})
    )
    _CACHE["last_result"] = res

    full = np.concatenate(
        [r["vout"].reshape(BP, H2, T) for r in res.results], axis=0
    )  # [104, 10, 2000]
    return np.ascontiguousarray(full.transpose(0, 2, 1)[:B].astype(np.float32))


# revision 39
# speedup vs baseline: 1.0446x; 1.0446x over previous
"""Trainium2 Bass kernel for LIFNet (leaky-integrator net, no spiking).

Math: the module is linear, and the leaky integration L (a causal LTI filter
along T) commutes with the per-timestep linear layers:

    V2 = L(L(batch @ W1^T) @ W2^T) = (L^2)(batch @ (W2 @ W1)^T)

with Wc = W2 @ W1 of shape [10, 784].  L^2 has impulse response
h[m] = beta^2 (m-1) alpha^(m-2) (m >= 2), which decays below f32 noise by
lag ~128, so the filter is applied as a banded blocked matmul with two
constant 128x128 blocks (intra-block R0, previous-block R1).

The kernel is HBM-bandwidth-bound (the batch read dominates), so:
  - x is pre-cast to fp16 on the host (quantization adds ~4e-4 rel err
    against a 2e-2 gate), halving DMA bytes vs f32.
  - The DRAM layout is per-partition contiguous per (b, t-half), giving
    14 KB descriptors; measured fastest issue pattern under all-8-core
    load is 26 half-b DMAs on the sync HWDGE queue (~250 GB/s/core; the
    f32 4 KB-descriptor version ran at ~207 GB/s).

Device work per core (13 b's, data-parallel over batch; groups of 4 b's
packed 32-partitions apart so downstream stages run 4 b's per instruction):
  - z^T = Wc @ x^T via PE matmuls (fp16, Wc chunks [112, 32] zero-padded,
    tile_position=(0, 32i) places b_i's output rows at psum partition 32i).
  - zp [128, 500] f32 -> zts [128, 2048] fp16 per-(b, tg) cast copies
    (scalar engine) issued right behind each b's matmul burst, so the
    next group's matmuls are not gated on a group-end copy barrier.
  - PE transpose per 128-t-block: [128, 128] -> tpsum fp16; DVE compacts
    the 4x10 used columns into zb slabs [128, 40].
  - V2^T[4 b's] per t'-block via two K=128 fp16 matmuls (R1 prev / R0 cur).
  - v2 [40, 2000] f32 DMA'd out per group on the scalar HWDGE queue.

PE-clock (HAM) management: the tensor engine re-throttles to 1.2 GHz
after any ~3.4us idle window, and transpose-mode matmuls do not count
as activity.  A few dummy matmuls pad each half-b burst and the
transpose bursts so the clock stays at 2.4 GHz.

Tail: the 1-b group is loaded and processed FIRST (its whole pipeline
hides under the other groups' loads); each group's t'-blocks 0-6 are
transposed + filtered as soon as its last b's first half lands; the
last group uses per-b transposes so after the final DMA byte only the
final b's own transposes + the filter remain serial.  Outputs are fp16
(host upcasts) and split per group into [0:896) / [896:2000) slices so
most of the write overlaps the filter.
"""

import sys

import numpy as np

for _p in ("/opt/trn_rl_repo",):
    if _p not in sys.path:
        sys.path.append(_p)

B, T, DIN, H1, H2 = 100, 2000, 784, 100, 10
ALPHA, BETA = 0.7, 0.3

NCORES = 8
BPAD = 104           # batch padded to 8 * 13
BP = BPAD // NCORES  # 13 b's per core
DC = 112             # d-chunk width (784 = 7 * 112), partition dim of x tiles
NDC = DIN // DC      # 7
MP = 32              # padded output rows per b (10 real + 22 zero)
TH = T // 2          # t-half
TG = 500             # t-columns per z-matmul group (one psum bank)
NTG = T // TG        # 4
TB = 128             # t'-block for the filter stage
NTB = (T + TB - 1) // TB   # 16
TPADF = NTB * TB     # 2048 free-dim padding for the z^T staging buffer
# Singleton first: its data arrives in the first two DMA slots and its
# whole pipeline overlaps the other groups' loads, so the kernel tail is
# only the last 4-group's second filter phase.
GROUPS = [(12, 1), (0, 4), (4, 4), (8, 4)]  # (first b, group size)
NWARM = 4            # dummy N=500 matmuls appended per half-b burst

_CACHE: dict = {}


def _filter_blocks() -> np.ndarray:
    """R = [R1 | R0] as [128, 256] fp16: rhs blocks for the filter matmuls.

    out[o, t'] += sum_tl z_block[tl, o] * R[tl, t'] with R[tl, t'] =
    h[lag], lag = (t' - tl) + 128 for R1 (z from previous t-block) and
    (t' - tl) for R0 (intra-block, strictly causal).
    """
    m = np.arange(512, dtype=np.float64)
    h = np.zeros(512)
    h[2:] = BETA * BETA * (m[2:] - 1.0) * ALPHA ** (m[2:] - 2.0)
    tl = np.arange(TB)[:, None]
    tp = np.arange(TB)[None, :]
    r1 = h[tp - tl + TB]
    lag0 = tp - tl
    r0 = np.where(lag0 >= 2, h[np.clip(lag0, 0, None)], 0.0)
    return np.concatenate([r1, r0], axis=1).astype(np.float16)


def _build(reps: int = 1):
    """Build + compile the per-core Bass kernel (shared by all 8 cores)."""
    from contextlib import ExitStack

    import concourse.tile as tile
    from concourse import bacc, mybir

    f16 = mybir.dt.float16
    f32 = mybir.dt.float32
    nc = bacc.Bacc(
        "TRN2", target_bir_lowering=False, debug=False, num_devices=NCORES
    )

    # per-partition layout per b: [half h][chunk c][t' in half] (7000 each)
    xT = nc.dram_tensor("xT", [BP, DC, NDC * T], f16, kind="ExternalInput")
    wct = nc.dram_tensor("wct", [DC, NDC * MP], f16, kind="ExternalInput")
    rh = nc.dram_tensor("rh", [TB, 2 * TB], f16, kind="ExternalInput")
    eye = nc.dram_tensor("eye", [TB, TB], f16, kind="ExternalInput")
    vout = nc.dram_tensor("vout", [BP * H2, T], f16, kind="ExternalOutput")

    with tile.TileContext(nc) as tc, ExitStack() as ctx:
        const = ctx.enter_context(tc.tile_pool(name="const", bufs=1))
        xpool = ctx.enter_context(tc.tile_pool(name="xp", bufs=10))
        ring = ctx.enter_context(tc.tile_pool(name="ring", bufs=1))
        zbp = ctx.enter_context(tc.tile_pool(name="zbp", bufs=2))
        vsb = ctx.enter_context(tc.tile_pool(name="vsb", bufs=2))
        zpsum = ctx.enter_context(tc.tile_pool(name="zps", bufs=1, space="PSUM"))
        tpsum = ctx.enter_context(tc.tile_pool(name="tps", bufs=2, space="PSUM"))
        vpsum = ctx.enter_context(tc.tile_pool(name="vps", bufs=1, space="PSUM"))
        dpsum = ctx.enter_context(tc.tile_pool(name="dps", bufs=1, space="PSUM"))

        # consts on the scalar HWDGE queue so they don't delay the first
        # x load on the sync queue
        wct_sb = const.tile([DC, NDC * MP], f16, tag="wct")
        nc.scalar.dma_start(wct_sb[:], wct.ap())
        rh_sb = const.tile([TB, 2 * TB], f16, tag="rh")
        nc.scalar.dma_start(rh_sb[:], rh.ap())
        eye_sb = const.tile([TB, TB], f16, tag="eye")
        nc.scalar.dma_start(eye_sb[:], eye.ap())

        # Two-deep manual ring: the t-pad cols (>=2000) of the z^T staging
        # tile must stay zero across groups, so memset only once.
        zts_ring = []
        for i in range(2):
            zt = ring.tile([TB, TPADF], f16, tag=f"zts{i}", name=f"zts{i}")
            nc.vector.memset(zt[:], 0.0)
            zts_ring.append(zt)

        def warm(xv, n=NWARM):
            """Dummy matmuls: count as PE activity for the HAM clock gate."""
            dmy = dpsum.tile([1, TG], f32, tag="dmy", name="dmy")
            for _ in range(n):
                nc.tensor.matmul(
                    dmy[:], wct_sb[:, 0:1], xv[:, 0, 0:TG],
                    start=True, stop=True,
                )

        def z_half(zp_tiles, xv, i, h, copy_rows, zts, nwarm=NWARM):
            """One half-b of stage-1 matmuls + its two zts cast copies."""
            for tg in (0, 1):
                zp = zp_tiles[2 * h + tg]
                for c in range(NDC):
                    nc.tensor.matmul(
                        zp[MP * i : MP * (i + 1), :],
                        wct_sb[:, c * MP : (c + 1) * MP],
                        xv[:, c, tg * TG : (tg + 1) * TG],
                        start=(c == 0),
                        stop=(c == NDC - 1),
                        tile_position=(0, MP * i),
                    )
            warm(xv, nwarm)
            r0, r1 = copy_rows
            for tg in (0, 1):
                gtg = 2 * h + tg
                nc.scalar.copy(
                    zts[r0:r1, gtg * TG : (gtg + 1) * TG],
                    zp_tiles[gtg][r0:r1, :],
                )

        def transposes(zts, zbv, G, xv_warm, j0, j1, per_b=None):
            """z^T -> zb for t'-blocks [j0, j1).

            per_b=i transposes only b_i's 32-partition band (so the last
            group's final b leaves just its own transposes for the tail).
            """
            for j in range(j0, j1):
                tp = tpsum.tile([TB, TB], f16, tag="tp", name="tp")
                if per_b is None:
                    nc.tensor.transpose(
                        tp[:], zts[:, j * TB : (j + 1) * TB], eye_sb[:]
                    )
                    tpv = tp[:].rearrange("p (gg o) -> p gg o", gg=4)
                    nc.vector.tensor_copy(
                        zbv[:, j, 0:G, :], tpv[:, 0:G, 0:H2]
                    )
                else:
                    i = per_b
                    nc.tensor.transpose(
                        tp[:, 0:MP],
                        zts[MP * i : MP * (i + 1), j * TB : (j + 1) * TB],
                        eye_sb[MP * i : MP * (i + 1), MP * i : MP * (i + 1)],
                        tile_position=(MP * i, 0),
                    )
                    nc.vector.tensor_copy(
                        zbv[:, j, i, :], tp[:, 0:H2]
                    )
                if j % 4 == 3 and xv_warm is not None:
                    # transpose-mode matmuls don't register as PE activity
                    # for the clock gate; sprinkle a real one
                    dmy = dpsum.tile([1, TG], f32, tag="dmy", name="dmy")
                    nc.tensor.matmul(
                        dmy[:], wct_sb[:, 0:1], xv_warm[:, 0, 0:TG],
                        start=True, stop=True,
                    )

        def stage23(zts, zb, zbv, v2, G, OG, xv_warm, j0, j1, skip_t=False):
            """Transpose + filter for t'-blocks [j0, j1)."""
            if not skip_t:
                transposes(zts, zbv, G, xv_warm, j0, j1)
            for j in range(j0, j1):
                vp = vpsum.tile([4 * H2, TB], f32, tag="vp", name="vp")
                n_mm = 2 if j > 0 else 1
                mm = 0
                for roff, jj in ((0, j - 1), (TB, j)):
                    if jj < 0:
                        continue
                    nc.tensor.matmul(
                        vp[0:OG, :],
                        zb[:, jj * 4 * H2 : jj * 4 * H2 + OG],
                        rh_sb[:, roff : roff + TB],
                        start=(mm == 0),
                        stop=(mm == n_mm - 1),
                    )
                    mm += 1
                w = min(TB, T - j * TB)
                nc.vector.tensor_copy(
                    v2[0:OG, j * TB : j * TB + w], vp[0:OG, 0:w]
                )

        for rep in range(reps):
          for g, (b0, G) in enumerate(GROUPS):
            zts = zts_ring[g % 2]
            last_grp = g == len(GROUPS) - 1

            zp_tiles = [
                zpsum.tile([TB, TG], f32, tag=f"zp{tg}", name=f"zp{tg}")
                for tg in range(NTG)
            ]
            zb = zbp.tile([TB, NTB * 4 * H2], f16, tag="zb")
            zbv = zb[:].rearrange("p (j gg o) -> p j gg o", j=NTB, gg=4)
            v2 = vsb.tile([4 * H2, T], f16, tag="v2")
            OG = H2 * G

            for i in range(G):
                b = b0 + i
                rows = (MP * i, MP * (i + 1))
                for h in range(2):
                    xt = xpool.tile([DC, NDC * TH], f16, tag="xt")
                    xv = xt[:].rearrange("p (c t) -> p c t", c=NDC)
                    nc.sync.dma_start(
                        xt[:],
                        xT.ap()[b, :, h * NDC * TH : (h + 1) * NDC * TH],
                    )
                    z_half(zp_tiles, xv, i, h, rows, zts)
                    if i == G - 1 and h == 0:
                        # t'-blocks 0-6 only need t < 896: transpose +
                        # filter them while the last half-b streams in
                        stage23(zts, zb, zbv, v2, G, OG, xv, 0, 7)
                        nc.scalar.dma_start(
                            vout.ap()[H2 * b0 : H2 * b0 + OG, 0 : 7 * TB],
                            v2[0:OG, 0 : 7 * TB],
                        )
                    if last_grp and h == 1:
                        # per-b transposes as each b's second half finishes:
                        # after the final DMA only the last b's own
                        # transposes and filter remain on the critical path
                        transposes(zts, zbv, G, xv, 7, NTB, per_b=i)
            stage23(zts, zb, zbv, v2, G, OG, xv, 7, NTB, skip_t=last_grp)
            # final slice on the sync queue: it is idle by now, while the
            # scalar engine is still draining copies ahead of its queue
            outq = nc.sync if g == len(GROUPS) - 1 else nc.scalar
            outq.dma_start(
                vout.ap()[H2 * b0 : H2 * b0 + OG, 7 * TB : T],
                v2[0:OG, 7 * TB : T],
            )

    nc.compile()
    return nc


def _prep_inputs(batch: np.ndarray, W1: np.ndarray, W2: np.ndarray):
    wc = (W2.astype(np.float64) @ W1.astype(np.float64)).astype(np.float32)
    # [112, 7, 32]: wct[p, c, o] = Wc[o, 112c + p] for o < 10, else 0
    wct = np.zeros((DC, NDC, MP), np.float16)
    wct[:, :, :H2] = wc.T.reshape(NDC, DC, H2).transpose(1, 0, 2)
    wct = np.ascontiguousarray(wct.reshape(DC, NDC * MP))
    rh = _filter_blocks()
    eye = np.eye(TB, dtype=np.float16)

    bp = np.zeros((BPAD, T, DIN), np.float16)
    bp[:B] = batch.astype(np.float16)
    # [8, 13, 112, 2, 7, 1000]: per-(b, half) partition runs of 14 KB
    xt = np.ascontiguousarray(
        bp.reshape(NCORES, BP, 2, TH, NDC, DC).transpose(0, 1, 5, 2, 4, 3)
    ).reshape(NCORES, BP, DC, NDC * T)
    return xt, wct, rh, eye


def kernel(batch: np.ndarray, W1: np.ndarray, W2: np.ndarray) -> np.ndarray:
    from concourse import bass_utils

    if "nc" not in _CACHE:
        _CACHE["nc"] = _build()
    nc = _CACHE["nc"]

    xt, wct, rh, eye = _prep_inputs(batch, W1, W2)
    in_maps = [
        {"xT": xt[i], "wct": wct, "rh": rh, "eye": eye} for i in range(NCORES)
    ]
    res = bass_utils.run_bass_kernel_spmd(
        nc, in_maps, core_ids=list(range(NCORES)), **_CACHE.get("run_kwargs", {})
    )
    _CACHE["last_result"] = res

    full = np.concatenate(
        [r["vout"].reshape(BP, H2, T) for r in res.results], axis=0
    )  # [104, 10, 2000]
    return np.ascontiguousarray(full.transpose(0, 2, 1)[:B].astype(np.float32))


# revision 43
# speedup vs baseline: 1.0499x; 1.0051x over previous
"""Trainium2 Bass kernel for LIFNet (leaky-integrator net, no spiking).

Math: the module is linear, and the leaky integration L (a causal LTI filter
along T) commutes with the per-timestep linear layers:

    V2 = L(L(batch @ W1^T) @ W2^T) = (L^2)(batch @ (W2 @ W1)^T)

with Wc = W2 @ W1 of shape [10, 784].  L^2 has impulse response
h[m] = beta^2 (m-1) alpha^(m-2) (m >= 2), which decays below f32 noise by
lag ~128, so the filter is applied as a banded blocked matmul with two
constant 128x128 blocks (intra-block R0, previous-block R1).

The kernel is HBM-bandwidth-bound (the batch read dominates), so:
  - x is pre-cast to fp16 on the host (quantization adds ~4e-4 rel err
    against a 2e-2 gate), halving DMA bytes vs f32.
  - The DRAM layout is per-partition contiguous per (b, t-half), giving
    14 KB descriptors; measured fastest issue pattern under all-8-core
    load is 26 half-b DMAs on the sync HWDGE queue (~250 GB/s/core; the
    f32 4 KB-descriptor version ran at ~207 GB/s).

Device work per core (13 b's, data-parallel over batch; groups of 4 b's
packed 32-partitions apart so downstream stages run 4 b's per instruction):
  - z^T = Wc @ x^T via PE matmuls (fp16, Wc chunks [112, 32] zero-padded,
    tile_position=(0, 32i) places b_i's output rows at psum partition 32i).
  - zp [128, 500] f32 -> zts [128, 2048] fp16 per-(b, tg) cast copies
    (scalar engine) issued right behind each b's matmul burst, so the
    next group's matmuls are not gated on a group-end copy barrier.
  - PE transpose per 128-t-block: [128, 128] -> tpsum fp16; DVE compacts
    the 4x10 used columns into zb slabs [128, 40].
  - V2^T[4 b's] per t'-block via two K=128 fp16 matmuls (R1 prev / R0 cur).
  - v2 [40, 2000] f32 DMA'd out per group on the scalar HWDGE queue.

PE-clock (HAM) management: the tensor engine re-throttles to 1.2 GHz
after any ~3.4us idle window, and transpose-mode matmuls do not count
as activity.  A few dummy matmuls pad each half-b burst and the
transpose bursts so the clock stays at 2.4 GHz.

Tail: the 1-b group is loaded and processed FIRST (its whole pipeline
hides under the other groups' loads); each group's t'-blocks 0-6 are
transposed + filtered as soon as its last b's first half lands; the
last group uses per-b transposes so after the final DMA byte only the
final b's own transposes + the filter remain serial.  Outputs are fp16
(host upcasts) and split per group into [0:896) / [896:2000) slices so
most of the write overlaps the filter.
"""

import sys

import numpy as np

for _p in ("/opt/trn_rl_repo",):
    if _p not in sys.path:
        sys.path.append(_p)

B, T, DIN, H1, H2 = 100, 2000, 784, 100, 10
ALPHA, BETA = 0.7, 0.3

NCORES = 8
BP = 12              # full b's per core; plus one shared half-b (T-split)
NEB = 9              # mini-half staging blocks: 1 halo + 8 t'-blocks
DC = 112             # d-chunk width (784 = 7 * 112), partition dim of x tiles
NDC = DIN // DC      # 7
MP = 32              # padded output rows per b (10 real + 22 zero)
TH = T // 2          # t-half
TG = 500             # t-columns per z-matmul group (one psum bank)
NTG = T // TG        # 4
TB = 128             # t'-block for the filter stage
NTB = (T + TB - 1) // TB   # 16
TPADF = NTB * TB     # 2048 free-dim padding for the z^T staging buffer
# The extra half-b is loaded and processed first: its pipeline hides
# under the full groups' loads.  Each core carries 12 full b's plus one
# half of a shared b (pure T-split balance: 100 b's over 8 cores).  The
# half is filtered with a preceding 128-col halo block -- all-zero for a
# first half (the integrator starts at rest), real x[872:1000) for a
# second half -- so the instruction stream stays SPMD-uniform.
GROUPS = [(0, 4), (4, 4), (8, 4)]  # (first b, group size)
NWARM = 4            # dummy N=500 matmuls appended per half-b burst

_CACHE: dict = {}


def _filter_blocks() -> np.ndarray:
    """R = [R1 | R0] as [128, 256] fp16: rhs blocks for the filter matmuls.

    out[o, t'] += sum_tl z_block[tl, o] * R[tl, t'] with R[tl, t'] =
    h[lag], lag = (t' - tl) + 128 for R1 (z from previous t-block) and
    (t' - tl) for R0 (intra-block, strictly causal).
    """
    m = np.arange(512, dtype=np.float64)
    h = np.zeros(512)
    h[2:] = BETA * BETA * (m[2:] - 1.0) * ALPHA ** (m[2:] - 2.0)
    tl = np.arange(TB)[:, None]
    tp = np.arange(TB)[None, :]
    r1 = h[tp - tl + TB]
    lag0 = tp - tl
    r0 = np.where(lag0 >= 2, h[np.clip(lag0, 0, None)], 0.0)
    return np.concatenate([r1, r0], axis=1).astype(np.float16)


def _build(reps: int = 1):
    """Build + compile the per-core Bass kernel (shared by all 8 cores)."""
    from contextlib import ExitStack

    import concourse.tile as tile
    from concourse import bacc, mybir

    f16 = mybir.dt.float16
    f32 = mybir.dt.float32
    nc = bacc.Bacc(
        "TRN2", target_bir_lowering=False, debug=False, num_devices=NCORES
    )

    # per-partition layout per b: [half h][chunk c][t' in half] (7000 each)
    xT = nc.dram_tensor("xT", [BP, DC, NDC * T], f16, kind="ExternalInput")
    wct = nc.dram_tensor("wct", [DC, NDC * MP], f16, kind="ExternalInput")
    rh = nc.dram_tensor("rh", [TB, 2 * TB], f16, kind="ExternalInput")
    eye = nc.dram_tensor("eye", [TB, TB], f16, kind="ExternalInput")
    vout = nc.dram_tensor("vout", [BP * H2, T], f16, kind="ExternalOutput")
    xE = nc.dram_tensor("xE", [DC, NDC * TH], f16, kind="ExternalInput")
    xH = nc.dram_tensor("xH", [DC, NDC * TB], f16, kind="ExternalInput")
    voutE = nc.dram_tensor("voutE", [H2, TH], f16, kind="ExternalOutput")

    with tile.TileContext(nc) as tc, ExitStack() as ctx:
        const = ctx.enter_context(tc.tile_pool(name="const", bufs=1))
        xpool = ctx.enter_context(tc.tile_pool(name="xp", bufs=10))
        ring = ctx.enter_context(tc.tile_pool(name="ring", bufs=1))
        zbp = ctx.enter_context(tc.tile_pool(name="zbp", bufs=2))
        vsb = ctx.enter_context(tc.tile_pool(name="vsb", bufs=2))
        zpsum = ctx.enter_context(tc.tile_pool(name="zps", bufs=1, space="PSUM"))
        tpsum = ctx.enter_context(tc.tile_pool(name="tps", bufs=2, space="PSUM"))
        vpsum = ctx.enter_context(tc.tile_pool(name="vps", bufs=1, space="PSUM"))
        dpsum = ctx.enter_context(tc.tile_pool(name="dps", bufs=1, space="PSUM"))

        # consts on the scalar HWDGE queue so they don't delay the first
        # x load on the sync queue
        wct_sb = const.tile([DC, NDC * MP], f16, tag="wct")
        nc.scalar.dma_start(wct_sb[:], wct.ap())
        rh_sb = const.tile([TB, 2 * TB], f16, tag="rh")
        nc.scalar.dma_start(rh_sb[:], rh.ap())
        eye_sb = const.tile([TB, TB], f16, tag="eye")
        nc.scalar.dma_start(eye_sb[:], eye.ap())

        # Two-deep manual ring: the t-pad cols (>=2000) of the z^T staging
        # tile must stay zero across groups, so memset only once.
        zts_ring = []
        for i in range(2):
            zt = ring.tile([TB, TPADF], f16, tag=f"zts{i}", name=f"zts{i}")
            nc.vector.memset(zt[:], 0.0)
            zts_ring.append(zt)

        def warm(xv, n=NWARM):
            """Dummy matmuls: count as PE activity for the HAM clock gate."""
            dmy = dpsum.tile([1, TG], f32, tag="dmy", name="dmy")
            for _ in range(n):
                nc.tensor.matmul(
                    dmy[:], wct_sb[:, 0:1], xv[:, 0, 0:TG],
                    start=True, stop=True,
                )

        def z_half(zp_tiles, xv, i, h, copy_rows, zts, nwarm=NWARM):
            """One half-b of stage-1 matmuls + its two zts cast copies."""
            for tg in (0, 1):
                zp = zp_tiles[2 * h + tg]
                for c in range(NDC):
                    nc.tensor.matmul(
                        zp[MP * i : MP * (i + 1), :],
                        wct_sb[:, c * MP : (c + 1) * MP],
                        xv[:, c, tg * TG : (tg + 1) * TG],
                        start=(c == 0),
                        stop=(c == NDC - 1),
                        tile_position=(0, MP * i),
                    )
            warm(xv, nwarm)
            r0, r1 = copy_rows
            for tg in (0, 1):
                gtg = 2 * h + tg
                nc.scalar.copy(
                    zts[r0:r1, gtg * TG : (gtg + 1) * TG],
                    zp_tiles[gtg][r0:r1, :],
                )

        def transposes(zts, zbv, G, xv_warm, j0, j1, per_b=None):
            """z^T -> zb for t'-blocks [j0, j1).

            per_b=i transposes only b_i's 32-partition band (so the last
            group's final b leaves just its own transposes for the tail).
            """
            for j in range(j0, j1):
                tp = tpsum.tile([TB, TB], f16, tag="tp", name="tp")
                if per_b is None:
                    nc.tensor.transpose(
                        tp[:], zts[:, j * TB : (j + 1) * TB], eye_sb[:]
                    )
                    tpv = tp[:].rearrange("p (gg o) -> p gg o", gg=4)
                    nc.vector.tensor_copy(
                        zbv[:, j, 0:G, :], tpv[:, 0:G, 0:H2]
                    )
                else:
                    i = per_b
                    nc.tensor.transpose(
                        tp[:, 0:MP],
                        zts[MP * i : MP * (i + 1), j * TB : (j + 1) * TB],
                        eye_sb[MP * i : MP * (i + 1), MP * i : MP * (i + 1)],
                        tile_position=(MP * i, 0),
                    )
                    nc.vector.tensor_copy(
                        zbv[:, j, i, :], tp[:, 0:H2]
                    )
                if j % 4 == 3 and xv_warm is not None:
                    # transpose-mode matmuls don't register as PE activity
                    # for the clock gate; sprinkle a real one
                    dmy = dpsum.tile([1, TG], f32, tag="dmy", name="dmy")
                    nc.tensor.matmul(
                        dmy[:], wct_sb[:, 0:1], xv_warm[:, 0, 0:TG],
                        start=True, stop=True,
                    )

        def stage23(zts, zb, zbv, v2, G, OG, xv_warm, j0, j1, skip_t=False):
            """Transpose + filter for t'-blocks [j0, j1)."""
            if not skip_t:
                transposes(zts, zbv, G, xv_warm, j0, j1)
            for j in range(j0, j1):
                vp = vpsum.tile([4 * H2, TB], f32, tag="vp", name="vp")
                n_mm = 2 if j > 0 else 1
                mm = 0
                for roff, jj in ((0, j - 1), (TB, j)):
                    if jj < 0:
                        continue
                    nc.tensor.matmul(
                        vp[0:OG, :],
                        zb[:, jj * 4 * H2 : jj * 4 * H2 + OG],
                        rh_sb[:, roff : roff + TB],
                        start=(mm == 0),
                        stop=(mm == n_mm - 1),
                    )
                    mm += 1
                w = min(TB, T - j * TB)
                nc.vector.tensor_copy(
                    v2[0:OG, j * TB : j * TB + w], vp[0:OG, 0:w]
                )

        def mini_half():
            """The shared half-b: halo block + 8 t'-blocks of 128.

            Staging layout (rows 0:32, one b at slot 0): mini-zts =
            [halo 128 | 1000 t | 24 zero-pad].  Filter block j takes R1
            from staging block j and R0 from staging block j+1; a zero
            halo makes this exact for a first half (rest state)."""
            ztm = ring.tile([TB, NEB * TB], f16, tag="ztm", name="ztm")
            nc.vector.memset(ztm[:], 0.0)
            xh = xpool.tile([DC, NDC * TB], f16, tag="xh", name="xh", bufs=1)
            nc.sync.dma_start(xh[:], xH.ap())
            xhv = xh[:].rearrange("p (c t) -> p c t", c=NDC)
            xe = xpool.tile([DC, NDC * TH], f16, tag="xt")
            nc.sync.dma_start(xe[:], xE.ap())
            xev = xe[:].rearrange("p (c t) -> p c t", c=NDC)
            # halo z (zeros in DRAM for an h0 half)
            zph = zpsum.tile([TB, TG], f32, tag="zp0", name="zph")
            for c in range(NDC):
                nc.tensor.matmul(
                    zph[0:MP, 0:TB], wct_sb[:, c * MP : (c + 1) * MP],
                    xhv[:, c, :], start=(c == 0), stop=(c == NDC - 1),
                    tile_position=(0, 0),
                )
            nc.scalar.copy(ztm[0:MP, 0:TB], zph[0:MP, 0:TB])
            for tg in (0, 1):
                zpe = zpsum.tile([TB, TG], f32, tag=f"zp{tg + 1}", name="zpe")
                for c in range(NDC):
                    nc.tensor.matmul(
                        zpe[0:MP, :], wct_sb[:, c * MP : (c + 1) * MP],
                        xev[:, c, tg * TG : (tg + 1) * TG],
                        start=(c == 0), stop=(c == NDC - 1),
                        tile_position=(0, 0),
                    )
                nc.scalar.copy(
                    ztm[0:MP, TB + tg * TG : TB + (tg + 1) * TG],
                    zpe[0:MP, :],
                )
            warm(xev)
            # transposes of the 9 staging blocks into zbm slabs of 10
            zbm = zbp.tile([TB, NEB * H2], f16, tag="zbm", name="zbm")
            for j in range(NEB):
                tp = tpsum.tile([TB, TB], f16, tag="tp", name="tp")
                nc.tensor.transpose(
                    tp[:, 0:MP], ztm[0:MP, j * TB : (j + 1) * TB],
                    eye_sb[0:MP, 0:MP], tile_position=(0, 0),
                )
                nc.vector.tensor_copy(
                    zbm[:, j * H2 : (j + 1) * H2], tp[:, 0:H2]
                )
                if j % 4 == 3:
                    warm(xev, 1)
            # filter: out-block j <- R1 x staging j + R0 x staging j+1
            v2e = vsb.tile([4 * H2, T], f16, tag="v2", name="v2e")
            for j in range(8):
                vp = vpsum.tile([4 * H2, TB], f32, tag="vp", name="vp")
                nc.tensor.matmul(
                    vp[0:H2, :], zbm[:, j * H2 : (j + 1) * H2],
                    rh_sb[:, 0:TB], start=True, stop=False,
                )
                nc.tensor.matmul(
                    vp[0:H2, :], zbm[:, (j + 1) * H2 : (j + 2) * H2],
                    rh_sb[:, TB : 2 * TB], start=False, stop=True,
                )
                w = min(TB, TH - j * TB)
                nc.vector.tensor_copy(
                    v2e[0:H2, j * TB : j * TB + w], vp[0:H2, 0:w]
                )
            nc.scalar.dma_start(voutE.ap(), v2e[0:H2, 0:TH])

        for rep in range(reps):
          mini_half()
          for g, (b0, G) in enumerate(GROUPS):
            zts = zts_ring[g % 2]
            last_grp = g == len(GROUPS) - 1

            zp_tiles = [
                zpsum.tile([TB, TG], f32, tag=f"zp{tg}", name=f"zp{tg}")
                for tg in range(NTG)
            ]
            zb = zbp.tile([TB, NTB * 4 * H2], f16, tag="zb")
            zbv = zb[:].rearrange("p (j gg o) -> p j gg o", j=NTB, gg=4)
            v2 = vsb.tile([4 * H2, T], f16, tag="v2")
            OG = H2 * G

            for i in range(G):
                b = b0 + i
                rows = (MP * i, MP * (i + 1))
                for h in range(2):
                    xt = xpool.tile([DC, NDC * TH], f16, tag="xt")
                    xv = xt[:].rearrange("p (c t) -> p c t", c=NDC)
                    nc.sync.dma_start(
                        xt[:],
                        xT.ap()[b, :, h * NDC * TH : (h + 1) * NDC * TH],
                    )
                    z_half(zp_tiles, xv, i, h, rows, zts)
                    if i == G - 1 and h == 0:
                        # t'-blocks 0-6 only need t < 896: transpose +
                        # filter them while the last half-b streams in
                        stage23(zts, zb, zbv, v2, G, OG, xv, 0, 7)
                        nc.scalar.dma_start(
                            vout.ap()[H2 * b0 : H2 * b0 + OG, 0 : 7 * TB],
                            v2[0:OG, 0 : 7 * TB],
                        )
                    if last_grp and h == 1:
                        # per-b transposes as each b's second half finishes:
                        # after the final DMA only the last b's own
                        # transposes and filter remain on the critical path
                        transposes(zts, zbv, G, xv, 7, NTB, per_b=i)
            stage23(zts, zb, zbv, v2, G, OG, xv, 7, NTB, skip_t=last_grp)
            # final slice on the sync queue: it is idle by now, while the
            # scalar engine is still draining copies ahead of its queue
            outq = nc.sync if g == len(GROUPS) - 1 else nc.scalar
            outq.dma_start(
                vout.ap()[H2 * b0 : H2 * b0 + OG, 7 * TB : T],
                v2[0:OG, 7 * TB : T],
            )

    nc.compile()
    return nc


def _prep_inputs(batch: np.ndarray, W1: np.ndarray, W2: np.ndarray):
    wc = (W2.astype(np.float64) @ W1.astype(np.float64)).astype(np.float32)
    # [112, 7, 32]: wct[p, c, o] = Wc[o, 112c + p] for o < 10, else 0
    wct = np.zeros((DC, NDC, MP), np.float16)
    wct[:, :, :H2] = wc.T.reshape(NDC, DC, H2).transpose(1, 0, 2)
    wct = np.ascontiguousarray(wct.reshape(DC, NDC * MP))
    rh = _filter_blocks()
    eye = np.eye(TB, dtype=np.float16)

    b16 = batch.astype(np.float16)
    # 96 full b's, 12 per core: [8, 12, 112, 2, 7, 1000] layout
    xt = np.ascontiguousarray(
        b16[: NCORES * BP]
        .reshape(NCORES, BP, 2, TH, NDC, DC)
        .transpose(0, 1, 5, 2, 4, 3)
    ).reshape(NCORES, BP, DC, NDC * T)
    # b's 96-99 split into halves, one per core, plus the h1 halo
    xe = np.empty((NCORES, DC, NDC * TH), np.float16)
    xh = np.zeros((NCORES, DC, NDC * TB), np.float16)
    for k in range(NCORES):
        eb, h = NCORES * BP + k // 2, k % 2
        xe[k] = (
            b16[eb, h * TH : (h + 1) * TH]
            .reshape(TH, NDC, DC).transpose(2, 1, 0).reshape(DC, NDC * TH)
        )
        if h == 1:
            xh[k] = (
                b16[eb, TH - TB : TH]
                .reshape(TB, NDC, DC).transpose(2, 1, 0).reshape(DC, NDC * TB)
            )
    return xt, xe, xh, wct, rh, eye


def kernel(batch: np.ndarray, W1: np.ndarray, W2: np.ndarray) -> np.ndarray:
    from concourse import bass_utils

    if "nc" not in _CACHE:
        _CACHE["nc"] = _build()
    nc = _CACHE["nc"]

    xt, xe, xh, wct, rh, eye = _prep_inputs(batch, W1, W2)
    in_maps = [
        {"xT": xt[i], "xE": xe[i], "xH": xh[i], "wct": wct, "rh": rh,
         "eye": eye}
        for i in range(NCORES)
    ]
    res = bass_utils.run_bass_kernel_spmd(
        nc, in_maps, core_ids=list(range(NCORES)), **_CACHE.get("run_kwargs", {})
    )
    _CACHE["last_result"] = res

    out = np.empty((B, H2, T), np.float32)
    for k, r in enumerate(res.results):
        out[BP * k : BP * (k + 1)] = r["vout"].reshape(BP, H2, T)
        eb, h = NCORES * BP + k // 2, k % 2
        out[eb, :, h * TH : (h + 1) * TH] = r["voutE"]
    return np.ascontiguousarray(out.transpose(0, 2, 1))


# revision 44
# speedup vs baseline: 1.0779x; 1.0266x over previous
"""Trainium2 Bass kernel for LIFNet (leaky-integrator net, no spiking).

Math: the module is linear, and the leaky integration L (a causal LTI filter
along T) commutes with the per-timestep linear layers:

    V2 = L(L(batch @ W1^T) @ W2^T) = (L^2)(batch @ (W2 @ W1)^T)

with Wc = W2 @ W1 of shape [10, 784].  L^2 has impulse response
h[m] = beta^2 (m-1) alpha^(m-2) (m >= 2), which decays below f32 noise by
lag ~128, so the filter is applied as a banded blocked matmul with two
constant 128x128 blocks (intra-block R0, previous-block R1).

The kernel is HBM-bandwidth-bound (the batch read dominates), so:
  - x is pre-cast to fp16 on the host (quantization adds ~4e-4 rel err
    against a 2e-2 gate), halving DMA bytes vs f32.
  - The DRAM layout is per-partition contiguous per (b, t-half), giving
    14 KB descriptors; measured fastest issue pattern under all-8-core
    load is 26 half-b DMAs on the sync HWDGE queue (~250 GB/s/core; the
    f32 4 KB-descriptor version ran at ~207 GB/s).

Device work per core (13 b's, data-parallel over batch; groups of 4 b's
packed 32-partitions apart so downstream stages run 4 b's per instruction):
  - z^T = Wc @ x^T via PE matmuls (fp16, Wc chunks [112, 32] zero-padded,
    tile_position=(0, 32i) places b_i's output rows at psum partition 32i).
  - zp [128, 500] f32 -> zts [128, 2048] fp16 per-(b, tg) cast copies
    (scalar engine) issued right behind each b's matmul burst, so the
    next group's matmuls are not gated on a group-end copy barrier.
  - PE transpose per 128-t-block: [128, 128] -> tpsum fp16; DVE compacts
    the 4x10 used columns into zb slabs [128, 40].
  - V2^T[4 b's] per t'-block via two K=128 fp16 matmuls (R1 prev / R0 cur).
  - v2 [40, 2000] f32 DMA'd out per group on the scalar HWDGE queue.

PE-clock (HAM) management: the tensor engine re-throttles to 1.2 GHz
after any ~3.4us idle window, and transpose-mode matmuls do not count
as activity.  A few dummy matmuls pad each half-b burst and the
transpose bursts so the clock stays at 2.4 GHz.

Tail: the 1-b group is loaded and processed FIRST (its whole pipeline
hides under the other groups' loads); each group's t'-blocks 0-6 are
transposed + filtered as soon as its last b's first half lands; the
last group uses per-b transposes so after the final DMA byte only the
final b's own transposes + the filter remain serial.  Outputs are fp16
(host upcasts) and split per group into [0:896) / [896:2000) slices so
most of the write overlaps the filter.
"""

import sys

import numpy as np

for _p in ("/opt/trn_rl_repo",):
    if _p not in sys.path:
        sys.path.append(_p)

B, T, DIN, H1, H2 = 100, 2000, 784, 100, 10
ALPHA, BETA = 0.7, 0.3

NCORES = 8
BP = 12              # full b's per core; plus one shared half-b (T-split)
NEB = 9              # mini-half staging blocks: 1 halo + 8 t'-blocks
DC = 112             # d-chunk width (784 = 7 * 112), partition dim of x tiles
NDC = DIN // DC      # 7
MP = 32              # padded output rows per b (10 real + 22 zero)
TH = T // 2          # t-half
TG = 500             # t-columns per z-matmul group (one psum bank)
NTG = T // TG        # 4
TB = 128             # t'-block for the filter stage
NTB = (T + TB - 1) // TB   # 16
TPADF = NTB * TB     # 2048 free-dim padding for the z^T staging buffer
# The extra half-b is loaded and processed first: its pipeline hides
# under the full groups' loads.  Each core carries 12 full b's plus one
# half of a shared b (pure T-split balance: 100 b's over 8 cores).  The
# half is filtered with a preceding 128-col halo block -- all-zero for a
# first half (the integrator starts at rest), real x[872:1000) for a
# second half -- so the instruction stream stays SPMD-uniform.
GROUPS = [(0, 4), (4, 4), (8, 4)]  # (first b, group size)
NWARM = 4            # dummy N=500 matmuls appended per half-b burst

_CACHE: dict = {}


def _filter_blocks() -> np.ndarray:
    """R = [R1 | R0] as [128, 256] fp16: rhs blocks for the filter matmuls.

    out[o, t'] += sum_tl z_block[tl, o] * R[tl, t'] with R[tl, t'] =
    h[lag], lag = (t' - tl) + 128 for R1 (z from previous t-block) and
    (t' - tl) for R0 (intra-block, strictly causal).
    """
    m = np.arange(512, dtype=np.float64)
    h = np.zeros(512)
    h[2:] = BETA * BETA * (m[2:] - 1.0) * ALPHA ** (m[2:] - 2.0)
    tl = np.arange(TB)[:, None]
    tp = np.arange(TB)[None, :]
    r1 = h[tp - tl + TB]
    lag0 = tp - tl
    r0 = np.where(lag0 >= 2, h[np.clip(lag0, 0, None)], 0.0)
    return np.concatenate([r1, r0], axis=1).astype(np.float16)


def _build(reps: int = 1):
    """Build + compile the per-core Bass kernel (shared by all 8 cores)."""
    from contextlib import ExitStack

    import concourse.tile as tile
    from concourse import bacc, mybir

    f16 = mybir.dt.float16
    f32 = mybir.dt.float32
    nc = bacc.Bacc(
        "TRN2", target_bir_lowering=False, debug=False, num_devices=NCORES
    )

    # per-partition layout per b: [half h][chunk c][t' in half] (7000 each)
    xT = nc.dram_tensor("xT", [BP, DC, NDC * T], f16, kind="ExternalInput")
    wct = nc.dram_tensor("wct", [DC, NDC * MP], f16, kind="ExternalInput")
    rh = nc.dram_tensor("rh", [TB, 2 * TB], f16, kind="ExternalInput")
    eye = nc.dram_tensor("eye", [TB, TB], f16, kind="ExternalInput")
    vout = nc.dram_tensor("vout", [BP * H2, T], f16, kind="ExternalOutput")
    xE = nc.dram_tensor("xE", [DC, NDC * TH], f16, kind="ExternalInput")
    xH = nc.dram_tensor("xH", [DC, NDC * TB], f16, kind="ExternalInput")
    voutE = nc.dram_tensor("voutE", [H2, TH], f16, kind="ExternalOutput")

    with tile.TileContext(nc) as tc, ExitStack() as ctx:
        const = ctx.enter_context(tc.tile_pool(name="const", bufs=1))
        xpool = ctx.enter_context(tc.tile_pool(name="xp", bufs=10))
        ring = ctx.enter_context(tc.tile_pool(name="ring", bufs=1))
        zbp = ctx.enter_context(tc.tile_pool(name="zbp", bufs=2))
        vsb = ctx.enter_context(tc.tile_pool(name="vsb", bufs=2))
        zpsum = ctx.enter_context(tc.tile_pool(name="zps", bufs=1, space="PSUM"))
        tpsum = ctx.enter_context(tc.tile_pool(name="tps", bufs=2, space="PSUM"))
        vpsum = ctx.enter_context(tc.tile_pool(name="vps", bufs=1, space="PSUM"))
        dpsum = ctx.enter_context(tc.tile_pool(name="dps", bufs=1, space="PSUM"))

        # consts on the scalar HWDGE queue so they don't delay the first
        # x load on the sync queue
        wct_sb = const.tile([DC, NDC * MP], f16, tag="wct")
        nc.scalar.dma_start(wct_sb[:], wct.ap())
        rh_sb = const.tile([TB, 2 * TB], f16, tag="rh")
        nc.scalar.dma_start(rh_sb[:], rh.ap())
        eye_sb = const.tile([TB, TB], f16, tag="eye")
        nc.scalar.dma_start(eye_sb[:], eye.ap())

        # Two-deep manual ring: the t-pad cols (>=2000) of the z^T staging
        # tile must stay zero across groups, so memset only once.
        zts_ring = []
        for i in range(2):
            zt = ring.tile([TB, TPADF], f16, tag=f"zts{i}", name=f"zts{i}")
            nc.vector.memset(zt[:], 0.0)
            zts_ring.append(zt)

        def warm(xv, n=NWARM):
            """Dummy matmuls: count as PE activity for the HAM clock gate."""
            dmy = dpsum.tile([1, TG], f32, tag="dmy", name="dmy")
            for _ in range(n):
                nc.tensor.matmul(
                    dmy[:], wct_sb[:, 0:1], xv[:, 0, 0:TG],
                    start=True, stop=True,
                )

        def z_half(zp_tiles, xv, i, h, copy_rows, zts, nwarm=NWARM):
            """One half-b of stage-1 matmuls + its two zts cast copies."""
            for tg in (0, 1):
                zp = zp_tiles[2 * h + tg]
                for c in range(NDC):
                    nc.tensor.matmul(
                        zp[MP * i : MP * (i + 1), :],
                        wct_sb[:, c * MP : (c + 1) * MP],
                        xv[:, c, tg * TG : (tg + 1) * TG],
                        start=(c == 0),
                        stop=(c == NDC - 1),
                        tile_position=(0, MP * i),
                    )
            warm(xv, nwarm)
            r0, r1 = copy_rows
            for tg in (0, 1):
                gtg = 2 * h + tg
                nc.scalar.copy(
                    zts[r0:r1, gtg * TG : (gtg + 1) * TG],
                    zp_tiles[gtg][r0:r1, :],
                )

        def transposes(zts, zbv, G, xv_warm, j0, j1, per_b=None):
            """z^T -> zb for t'-blocks [j0, j1).

            per_b=i transposes only b_i's 32-partition band (so the last
            group's final b leaves just its own transposes for the tail).
            """
            for j in range(j0, j1):
                tp = tpsum.tile([TB, TB], f16, tag="tp", name="tp")
                if per_b is None:
                    nc.tensor.transpose(
                        tp[:], zts[:, j * TB : (j + 1) * TB], eye_sb[:]
                    )
                    tpv = tp[:].rearrange("p (gg o) -> p gg o", gg=4)
                    nc.vector.tensor_copy(
                        zbv[:, j, 0:G, :], tpv[:, 0:G, 0:H2]
                    )
                else:
                    i = per_b
                    nc.tensor.transpose(
                        tp[:, 0:MP],
                        zts[MP * i : MP * (i + 1), j * TB : (j + 1) * TB],
                        eye_sb[MP * i : MP * (i + 1), MP * i : MP * (i + 1)],
                        tile_position=(MP * i, 0),
                    )
                    nc.vector.tensor_copy(
                        zbv[:, j, i, :], tp[:, 0:H2]
                    )
                if j % 4 == 3 and xv_warm is not None:
                    # transpose-mode matmuls don't register as PE activity
                    # for the clock gate; sprinkle a real one
                    dmy = dpsum.tile([1, TG], f32, tag="dmy", name="dmy")
                    nc.tensor.matmul(
                        dmy[:], wct_sb[:, 0:1], xv_warm[:, 0, 0:TG],
                        start=True, stop=True,
                    )

        def stage23(zts, zb, zbv, v2, G, OG, xv_warm, j0, j1, skip_t=False):
            """Transpose + filter for t'-blocks [j0, j1)."""
            if not skip_t:
                transposes(zts, zbv, G, xv_warm, j0, j1)
            for j in range(j0, j1):
                vp = vpsum.tile([4 * H2, TB], f32, tag="vp", name="vp")
                n_mm = 2 if j > 0 else 1
                mm = 0
                for roff, jj in ((0, j - 1), (TB, j)):
                    if jj < 0:
                        continue
                    nc.tensor.matmul(
                        vp[0:OG, :],
                        zb[:, jj * 4 * H2 : jj * 4 * H2 + OG],
                        rh_sb[:, roff : roff + TB],
                        start=(mm == 0),
                        stop=(mm == n_mm - 1),
                    )
                    mm += 1
                w = min(TB, T - j * TB)
                nc.vector.tensor_copy(
                    v2[0:OG, j * TB : j * TB + w], vp[0:OG, 0:w]
                )

        def mini_half():
            """The shared half-b: halo block + 8 t'-blocks of 128.

            Staging layout (rows 0:32, one b at slot 0): mini-zts =
            [halo 128 | 1000 t | 24 zero-pad].  Filter block j takes R1
            from staging block j and R0 from staging block j+1; a zero
            halo makes this exact for a first half (rest state)."""
            ztm = ring.tile([TB, NEB * TB], f16, tag="ztm", name="ztm")
            nc.vector.memset(ztm[:], 0.0)
            xh = xpool.tile([DC, NDC * TB], f16, tag="xh", name="xh", bufs=1)
            nc.sync.dma_start(xh[:], xH.ap())
            xhv = xh[:].rearrange("p (c t) -> p c t", c=NDC)
            xe = xpool.tile([DC, NDC * TH], f16, tag="xt")
            nc.sync.dma_start(xe[:], xE.ap())
            xev = xe[:].rearrange("p (c t) -> p c t", c=NDC)
            # halo z (zeros in DRAM for an h0 half)
            zph = zpsum.tile([TB, TG], f32, tag="zp0", name="zph")
            for c in range(NDC):
                nc.tensor.matmul(
                    zph[0:MP, 0:TB], wct_sb[:, c * MP : (c + 1) * MP],
                    xhv[:, c, :], start=(c == 0), stop=(c == NDC - 1),
                    tile_position=(0, 0),
                )
            nc.scalar.copy(ztm[0:MP, 0:TB], zph[0:MP, 0:TB])
            for tg in (0, 1):
                zpe = zpsum.tile([TB, TG], f32, tag=f"zp{tg + 1}", name="zpe")
                for c in range(NDC):
                    nc.tensor.matmul(
                        zpe[0:MP, :], wct_sb[:, c * MP : (c + 1) * MP],
                        xev[:, c, tg * TG : (tg + 1) * TG],
                        start=(c == 0), stop=(c == NDC - 1),
                        tile_position=(0, 0),
                    )
                nc.scalar.copy(
                    ztm[0:MP, TB + tg * TG : TB + (tg + 1) * TG],
                    zpe[0:MP, :],
                )
            warm(xev)
            # transposes of the 9 staging blocks into zbm slabs of 10
            zbm = zbp.tile([TB, NEB * H2], f16, tag="zbm", name="zbm")
            for j in range(NEB):
                tp = tpsum.tile([TB, TB], f16, tag="tp", name="tp")
                nc.tensor.transpose(
                    tp[:, 0:MP], ztm[0:MP, j * TB : (j + 1) * TB],
                    eye_sb[0:MP, 0:MP], tile_position=(0, 0),
                )
                nc.vector.tensor_copy(
                    zbm[:, j * H2 : (j + 1) * H2], tp[:, 0:H2]
                )
                if j % 4 == 3:
                    warm(xev, 1)
            # filter: out-block j <- R1 x staging j + R0 x staging j+1
            v2e = vsb.tile([4 * H2, T], f16, tag="v2", name="v2e")
            for j in range(8):
                vp = vpsum.tile([4 * H2, TB], f32, tag="vp", name="vp")
                nc.tensor.matmul(
                    vp[0:H2, :], zbm[:, j * H2 : (j + 1) * H2],
                    rh_sb[:, 0:TB], start=True, stop=False,
                )
                nc.tensor.matmul(
                    vp[0:H2, :], zbm[:, (j + 1) * H2 : (j + 2) * H2],
                    rh_sb[:, TB : 2 * TB], start=False, stop=True,
                )
                w = min(TB, TH - j * TB)
                nc.vector.tensor_copy(
                    v2e[0:H2, j * TB : j * TB + w], vp[0:H2, 0:w]
                )
            nc.scalar.dma_start(voutE.ap(), v2e[0:H2, 0:TH])

        for rep in range(reps):
          mini_half()
          for g, (b0, G) in enumerate(GROUPS):
            zts = zts_ring[g % 2]
            last_grp = g == len(GROUPS) - 1

            zp_tiles = [
                zpsum.tile([TB, TG], f32, tag=f"zp{tg}", name=f"zp{tg}")
                for tg in range(NTG)
            ]
            zb = zbp.tile([TB, NTB * 4 * H2], f16, tag="zb")
            zbv = zb[:].rearrange("p (j gg o) -> p j gg o", j=NTB, gg=4)
            v2 = vsb.tile([4 * H2, T], f16, tag="v2")
            OG = H2 * G

            for i in range(G):
                b = b0 + i
                rows = (MP * i, MP * (i + 1))
                for h in range(2):
                    xt = xpool.tile([DC, NDC * TH], f16, tag="xt")
                    xv = xt[:].rearrange("p (c t) -> p c t", c=NDC)
                    nc.sync.dma_start(
                        xt[:],
                        xT.ap()[b, :, h * NDC * TH : (h + 1) * NDC * TH],
                    )
                    z_half(zp_tiles, xv, i, h, rows, zts)
                    if i == G - 1 and h == 0:
                        # t'-blocks 0-6 only need t < 896: transpose +
                        # filter them while the last half-b streams in
                        stage23(zts, zb, zbv, v2, G, OG, xv, 0, 7)
                        nc.scalar.dma_start(
                            vout.ap()[H2 * b0 : H2 * b0 + OG, 0 : 7 * TB],
                            v2[0:OG, 0 : 7 * TB],
                        )
                    if last_grp and h == 1:
                        # per-b transposes as each b's second half finishes:
                        # after the final DMA only the last b's own
                        # transposes and filter remain on the critical path
                        transposes(zts, zbv, G, xv, 7, NTB, per_b=i)
            if last_grp:
                # split the filter + writeback once more so the [896:1408)
                # slice flies while blocks 11-15 are still filtering; the
                # final ~600-col slice goes on the idle sync queue
                stage23(zts, zb, zbv, v2, G, OG, xv, 7, 11, skip_t=True)
                nc.scalar.dma_start(
                    vout.ap()[H2 * b0 : H2 * b0 + OG, 7 * TB : 11 * TB],
                    v2[0:OG, 7 * TB : 11 * TB],
                )
                stage23(zts, zb, zbv, v2, G, OG, xv, 11, NTB, skip_t=True)
                nc.sync.dma_start(
                    vout.ap()[H2 * b0 : H2 * b0 + OG, 11 * TB : T],
                    v2[0:OG, 11 * TB : T],
                )
            else:
                stage23(zts, zb, zbv, v2, G, OG, xv, 7, NTB)
                nc.scalar.dma_start(
                    vout.ap()[H2 * b0 : H2 * b0 + OG, 7 * TB : T],
                    v2[0:OG, 7 * TB : T],
                )

    nc.compile()
    return nc


def _prep_inputs(batch: np.ndarray, W1: np.ndarray, W2: np.ndarray):
    wc = (W2.astype(np.float64) @ W1.astype(np.float64)).astype(np.float32)
    # [112, 7, 32]: wct[p, c, o] = Wc[o, 112c + p] for o < 10, else 0
    wct = np.zeros((DC, NDC, MP), np.float16)
    wct[:, :, :H2] = wc.T.reshape(NDC, DC, H2).transpose(1, 0, 2)
    wct = np.ascontiguousarray(wct.reshape(DC, NDC * MP))
    rh = _filter_blocks()
    eye = np.eye(TB, dtype=np.float16)

    b16 = batch.astype(np.float16)
    # 96 full b's, 12 per core: [8, 12, 112, 2, 7, 1000] layout
    xt = np.ascontiguousarray(
        b16[: NCORES * BP]
        .reshape(NCORES, BP, 2, TH, NDC, DC)
        .transpose(0, 1, 5, 2, 4, 3)
    ).reshape(NCORES, BP, DC, NDC * T)
    # b's 96-99 split into halves, one per core, plus the h1 halo
    xe = np.empty((NCORES, DC, NDC * TH), np.float16)
    xh = np.zeros((NCORES, DC, NDC * TB), np.float16)
    for k in range(NCORES):
        eb, h = NCORES * BP + k // 2, k % 2
        xe[k] = (
            b16[eb, h * TH : (h + 1) * TH]
            .reshape(TH, NDC, DC).transpose(2, 1, 0).reshape(DC, NDC * TH)
        )
        if h == 1:
            xh[k] = (
                b16[eb, TH - TB : TH]
                .reshape(TB, NDC, DC).transpose(2, 1, 0).reshape(DC, NDC * TB)
            )
    return xt, xe, xh, wct, rh, eye


def kernel(batch: np.ndarray, W1: np.ndarray, W2: np.ndarray) -> np.ndarray:
    from concourse import bass_utils

    if "nc" not in _CACHE:
        _CACHE["nc"] = _build()
    nc = _CACHE["nc"]

    xt, xe, xh, wct, rh, eye = _prep_inputs(batch, W1, W2)
    in_maps = [
        {"xT": xt[i], "xE": xe[i], "xH": xh[i], "wct": wct, "rh": rh,
         "eye": eye}
        for i in range(NCORES)
    ]
    res = bass_utils.run_bass_kernel_spmd(
        nc, in_maps, core_ids=list(range(NCORES)), **_CACHE.get("run_kwargs", {})
    )
    _CACHE["last_result"] = res

    out = np.empty((B, H2, T), np.float32)
    for k, r in enumerate(res.results):
        out[BP * k : BP * (k + 1)] = r["vout"].reshape(BP, H2, T)
        eb, h = NCORES * BP + k // 2, k % 2
        out[eb, :, h * TH : (h + 1) * TH] = r["voutE"]
    return np.ascontiguousarray(out.transpose(0, 2, 1))
